# revision 33
# baseline (speedup 1.0000x reference)
"""Trainium2 Bass kernel for BoxMultiHeadedAttention (B=8, N=512, D=512, H=8).

Sharding: data-parallel over batch - each of the 8 NeuronCores computes one
batch element end-to-end; weights replicated; no collectives.

Per-core pipeline (transposed-attention layout [j(part), i(free)]):
  P1: SWDGE casting loads (f32->bf16) of x/W; xT via DMA-transpose;
      q/k/v projections on PE; qT/kT evicted on ACT (bias via Identity).
  P2: ln fields: d on ACT (Identity+bias), |d|*iw via DVE stt(abs_max,mult),
      Ln on ACT, clip on DVE -> dxy2 f32.
  P3: dw/dh banks (angle-addition separable, baseline scheme).
  P4 per (rb,g): selector matmuls (f32) give t=alpha*dx/2pi; magic-round
      fold (2 DVE ops); s2=sin(pi f), ssin=sin(2pi f) on ACT (the double
      angle cos(2pi t)=1-2 s2^2 removes both DVE products); sq=s2*s2 on
      Pool; WG contraction on PE (bf16); A=relu(oj*wgp+oj*bG') on ACT
      (obj fold via per-partition scale AP); U=(A-oj)*oi via stt;
      U-slab written to DRAM in the per-head-contiguous layout.
  P5a (overlapped with P4 of next rb): scores st=kT^T qT on PE, E=exp on
      ACT (mask+shift bias), buffered.
  P5b: tt=(U+1)*E in ONE stt; row sums + AV on PE (accumulated).
  P6: reciprocal, DRAM-bounce broadcast of 1/s, AV scale, final linear.
"""
import math
import numpy as np
from contextlib import ExitStack

import ml_dtypes
import concourse.bass as bass
import concourse.mybir as mybir
import concourse.tile as tile
from concourse.bass_utils import run_bass_kernel_spmd

F32 = mybir.dt.float32
BF16 = mybir.dt.bfloat16
AF = mybir.ActivationFunctionType
ALU = mybir.AluOpType

B, N, D, H = 8, 512, 512, 8
DK = D // H          # 64
P = 128
NRB = N // P         # 4
NG = 8
GM = 16
WAVE_LEN = 1000.0
MAGIC = 12582912.0
CLIP = float(math.log(0.001))
ESHIFT = -6.0
PI_ = float(math.pi)
TWO_PI = float(2.0 * math.pi)

_alphas = (100.0 / (WAVE_LEN ** (np.arange(8) / 8.0))).astype(np.float64)


def _fix_waits(nc):
    """walrus accepts only ONE sync-wait per ISA instruction; DMA-transpose
    accepts none. Hoist extras onto NoOps inserted before the instruction."""
    n_fix = 0
    for blk in nc.main_func.blocks:
        insts = list(blk.instructions)
        out, dirty = [], False
        for inst in insts:
            si = inst.sync_info
            waits = list(si.on_wait) if si is not None else []
            keep = 0 if isinstance(inst, mybir.InstDmaTransposeAnt) else 1
            if len(waits) > keep:
                hoist = waits if keep == 0 else waits[:-1]
                for kk, w in enumerate(hoist):
                    out.append(mybir.InstNoOp(
                        name=f"I-waitfix-{n_fix}-{kk}", engine=inst.engine,
                        sync_info=mybir.SyncInfo(on_wait=[w], on_update=[])))
                rest = [] if keep == 0 else [waits[-1]]
                inst.sync_info = mybir.SyncInfo(
                    on_wait=rest, on_update=list(si.on_update))
                n_fix += 1
                dirty = True
            out.append(inst)
        if dirty:
            blk.instructions = out
    return n_fix


def _selector_const():
    # SELAP[64*W + q*16 + m, q, m*8 + j] = alpha_j/(2*pi)
    selap = np.zeros((P, 4, P), dtype=np.float32)
    for W in range(2):
        for q in range(4):
            for m in range(GM):
                for j in range(8):
                    selap[64 * W + q * 16 + m, q, m * 8 + j] = \
                        _alphas[j] / (2.0 * math.pi)
    return selap


def _onehot8():
    oh = np.zeros((P, H, H), dtype=np.float32)
    for h in range(H):
        oh[:, h, h] = 1.0
    return oh


def _wg_consts(WG, bG):
    out = {}
    # features: ssin (weight 1*WGs), sq=s2^2 (weight -2*WGc, const +WGc)
    gmap = [lambda j: j, lambda j: 32 + j, lambda j: 8 + j, lambda j: 40 + j]
    gscl = [1.0, -2.0, 1.0, -2.0]
    wblk = np.zeros((4, P, P), dtype=np.float32)
    for c in range(4):
        for m in range(GM):
            for j in range(8):
                for h in range(H):
                    wblk[c, m * 8 + j, h * GM + m] = \
                        gscl[c] * WG[h, gmap[c](j)]
    out["WBLK"] = wblk

    acol = np.zeros((64, 1), np.float32)
    pcol_m = np.zeros((64, 1), np.float32)
    pcol_n = np.zeros((64, 1), np.float32)
    w1 = np.zeros((64, H), np.float32)
    for f in range(2):
        for j in range(8):
            gs = 16 + 8 * f + j
            gc = 48 + 8 * f + j
            a = _alphas[j] / (4.0 * math.pi)
            for t in range(4):
                k = (f * 8 + j) * 4 + t
                acol[k, 0] = a
                pcol_m[k, 0] = 0.25 if t in (0, 2) else 0.0
                if t == 0:
                    pcol_n[k, 0] = 0.0; w1[k] = WG[:, gs]
                elif t == 1:
                    pcol_n[k, 0] = 0.75; w1[k] = WG[:, gs]   # -cos -> +pi
                elif t == 2:
                    pcol_n[k, 0] = 0.25; w1[k] = WG[:, gc]
                else:
                    pcol_n[k, 0] = 0.0; w1[k] = WG[:, gc]
    out["PCOLPK"] = np.concatenate([acol, pcol_m, pcol_n], axis=1)  # [64,3]
    out["W1E"] = np.repeat(w1, GM, axis=1).astype(np.float32)
    # bG' = bG + sum_j (WGc_x + WGc_y)  (the "+1" of cos = 1 - 2 sin^2)
    bg2 = bG.astype(np.float64) + WG[:, 32:48].sum(axis=1)
    out["BG2"] = bg2
    out["BGCOL"] = np.repeat(bg2.astype(np.float32), GM)[:, None]
    return out


def _host_prep(inputs):
    q = np.asarray(inputs["input_query"], np.float32)
    k = np.asarray(inputs["input_key"], np.float32)
    v = np.asarray(inputs["input_value"], np.float32)
    box = np.asarray(inputs["input_box"], np.float32)
    mask = np.asarray(inputs["mask"])
    nobj = np.asarray(inputs["not_objects"])
    WG = np.asarray(inputs["WG"], np.float32)
    bG = np.asarray(inputs["bG"], np.float32)
    wgc = _wg_consts(WG, bG)
    bg2 = wgc["BG2"]

    x_min, y_min, x_max, y_max = [box[..., i] for i in range(4)]
    cx = (x_min + x_max) * 0.5
    cy = (y_min + y_max) * 0.5
    ww = x_max - x_min + 1.0
    hh = y_max - y_min + 1.0

    maskcol = (np.where(mask == 0, -1e9, 0.0) + ESHIFT).astype(np.float32)
    obj = (1.0 - nobj.astype(np.float32)).astype(np.float32)

    shared = {
        "Wq": np.asarray(inputs["Wq"], np.float32),
        "Wk": np.asarray(inputs["Wk"], np.float32),
        "Wv": np.asarray(inputs["Wv"], np.float32),
        "Wo": np.asarray(inputs["Wo"], np.float32),
        "bvrow": np.asarray(inputs["bv"], np.float32),
        "borow": np.asarray(inputs["bo"], np.float32),
        "SELAP": _selector_const(),
        "ONEHOT8": _onehot8().astype(ml_dtypes.bfloat16),
        "WBLK": wgc["WBLK"], "W1E": wgc["W1E"],
        "PCOLPK": wgc["PCOLPK"], "BGCOL": wgc["BGCOL"],
    }
    bq = np.asarray(inputs["bq"], np.float32).reshape(NRB, P).T
    bk8 = (np.asarray(inputs["bk"], np.float32) / 8.0).reshape(NRB, P).T

    in_maps = []
    for b in range(B):
        # BCROWS: cx, cy, 1/w, 1/h, 2ln w, 2ln h
        bcrows = np.stack([
            cx[b], cy[b], 1.0 / ww[b], 1.0 / hh[b],
            2.0 * np.log(ww[b]), 2.0 * np.log(hh[b])]).astype(np.float32)
        # colpack [P, 84]: negcx, negcy, mcol, bq, bk/8, ojp, ojpbg
        # OJP[p=(h,m), k=rb*8+g] = obj at j = rb*128 + 64*(g//4) + 16*(g%4) + m
        ojp = np.zeros((P, 32), np.float32)
        for rb in range(NRB):
            for g in range(NG):
                jb = rb * P + 64 * (g // 4) + 16 * (g % 4)
                colv = obj[b, jb:jb + GM]          # [16]
                ojp[:, rb * 8 + g] = np.tile(colv, H)
        ojpbg = (ojp * np.repeat(bg2, GM)[:, None]).astype(np.float32)
        colpack = np.concatenate([
            -cx[b].reshape(NRB, P).T, -cy[b].reshape(NRB, P).T,
            maskcol[b].reshape(NRB, P).T, bq, bk8,
            ojp.astype(np.float32), ojpbg], axis=1).astype(np.float32)
        m = dict(shared)
        m.update({
            "xq": q[b].copy(), "xk": k[b].copy(), "xv": v[b].copy(),
            "BCROWS": bcrows, "COLPACK": colpack,
            "objrow_bf": obj[b].astype(ml_dtypes.bfloat16)[None, :],
        })
        in_maps.append(m)
    return in_maps


def build_nc():
    nc = bass.Bass()

    def dp(name, shape, dt=F32):
        return nc.declare_dram_parameter(name, list(shape), dt, isOutput=False)

    xq = dp("xq", (N, D)); xk = dp("xk", (N, D)); xv = dp("xv", (N, D))
    Wq = dp("Wq", (D, D)); Wk = dp("Wk", (D, D)); Wv = dp("Wv", (D, D))
    Wo = dp("Wo", (D, D))
    bvrow = dp("bvrow", (D,)); borow = dp("borow", (D,))
    BCROWS = dp("BCROWS", (6, N)); COLPACK = dp("COLPACK", (P, 84))
    objrow_bf = dp("objrow_bf", (1, N), BF16)
    SELAP = dp("SELAP", (P, 4, P))
    ONEHOT8 = dp("ONEHOT8", (P, H, H), BF16)
    WBLK = dp("WBLK", (4, P, P)); W1E = dp("W1E", (64, P))
    PCOLPK = dp("PCOLPK", (64, 3)); BGCOL = dp("BGCOL", (P, 1))
    out = nc.declare_dram_parameter("out", [N, D], F32, isOutput=True)
    wgd_dram = nc.dram_tensor("wgd_scratch", [NRB, H, P, N], BF16)
    rs_dram = nc.dram_tensor("rs_scratch", [H, N], F32)

    with ExitStack() as ctx:
        tc = ctx.enter_context(tile.TileContext(nc))
        const = ctx.enter_context(tc.tile_pool(name="const", bufs=1))
        persist = ctx.enter_context(tc.tile_pool(name="persist", bufs=1))

        # ---------------- constants ----------------
        colpk_t = const.tile([P, 84], F32, tag="colpk")
        nc.sync.dma_start(colpk_t[:], COLPACK[:])
        negcx = colpk_t[:, 0:4]; negcy = colpk_t[:, 4:8]
        mcol_t = colpk_t[:, 8:12]; bq_t = colpk_t[:, 12:16]
        bk8_t = colpk_t[:, 16:20]
        ojp_t = colpk_t[:, 20:52]
        ojpbg_t = colpk_t[:, 52:84]
        selap_t = const.tile([P, 4, P], F32, tag="selap")
        nc.sync.dma_start(selap_t[:], SELAP[:])
        pcolpk_t = const.tile([64, 3], F32, tag="pcolpk")
        nc.sync.dma_start(pcolpk_t[:], PCOLPK[:])
        acol_t = pcolpk_t[:, 0:1]
        pcolm_t = pcolpk_t[:, 1:2]; pcoln_t = pcolpk_t[:, 2:3]
        # broadcast rows: phase 2 needs them immediately
        cxbc = const.tile([P, N], F32, tag="cxbc")
        nc.sync.dma_start(cxbc[:], BCROWS[0:1, :].to_broadcast((P, N)))
        cybc = const.tile([P, N], F32, tag="cybc")
        nc.sync.dma_start(cybc[:], BCROWS[1:2, :].to_broadcast((P, N)))
        iwbc = const.tile([P, N], F32, tag="iwbc")
        nc.sync.dma_start(iwbc[:], BCROWS[2:3, :].to_broadcast((P, N)))
        ihbc = const.tile([P, N], F32, tag="ihbc")
        nc.sync.dma_start(ihbc[:], BCROWS[3:4, :].to_broadcast((P, N)))
        l2whbc = const.tile([64, N], F32, tag="l2whbc")
        nc.sync.dma_start(l2whbc[0:32, :],
                          BCROWS[4:5, :].to_broadcast((32, N)))
        nc.sync.dma_start(l2whbc[32:64, :],
                          BCROWS[5:6, :].to_broadcast((32, N)))
        w1e_f = const.tile([64, P], F32, tag="w1e")
        nc.sync.dma_start(w1e_f[:], W1E[:])
        objbc = const.tile([P, N], BF16, tag="objbc")
        nc.sync.dma_start(objbc[:], objrow_bf[0:1, :].to_broadcast((P, N)))
        oh8_t = const.tile([P, H, H], BF16, tag="oh8")
        nc.sync.dma_start(oh8_t[:], ONEHOT8[:])
        bvbc = const.tile([P, D], F32, tag="bvbc")
        nc.sync.dma_start(bvbc[:], bvrow[None, :].to_broadcast((P, D)))
        bobc = const.tile([P, D], F32, tag="bobc")
        nc.sync.dma_start(bobc[:], borow[None, :].to_broadcast((P, D)))
        # SWDGE casting loads: wblk bf16 first (p4 needs it), then inputs;
        # weight loads are emitted later (after p4 rb0) to keep the Pool
        # queue clear for the first sq ops.
        wblk_t = const.tile([P, 4, P], BF16, tag="wblkb")
        nc.gpsimd.dma_start(wblk_t[:], WBLK.rearrange("c p d -> p c d"))
        xq_sb = persist.tile([P, NRB, D], BF16, tag="xq_sb")
        xk_sb = persist.tile([P, NRB, D], BF16, tag="xk_sb")
        xv_sb = persist.tile([P, NRB, D], BF16, tag="xv_sb")
        wq_b = persist.tile([P, NRB, D], BF16, tag="wqb")
        wk_b = persist.tile([P, NRB, D], BF16, tag="wkb")
        wv_b = persist.tile([P, NRB, D], BF16, tag="wvb")
        wo_b = persist.tile([P, NRB, D], BF16, tag="wob")
        magic_col = const.tile([P, 1], F32, tag="magic_col")
        nc.vector.memset(magic_col[:], MAGIC)
        bgcol_t = const.tile([P, 1], F32, tag="bgcol")
        nc.sync.dma_start(bgcol_t[:], BGCOL[:])
        bgm1_t = const.tile([P, 1], F32, tag="bgm1")
        nc.vector.tensor_scalar(bgm1_t[:], bgcol_t[:], -1.0, None, ALU.add)

        # ---------------- phase 2: ln fields ----------------
        dxy2 = []
        for _rb in range(NRB):
            dxy2_rb = persist.tile([P, 2, N], F32, tag=f"dxy2_{_rb}")
            dxy2.append(dxy2_rb)
        with tc.tile_pool(name="work2", bufs=2) as work2:
            for rb in range(NRB):
                for (ci, cbc, ncol, ibc) in ((0, cxbc, negcx, iwbc),
                                             (1, cybc, negcy, ihbc)):
                    d_ = work2.tile([P, N], F32, tag="geo_d")
                    nc.scalar.activation(d_[:], cbc[:], AF.Abs,
                                         bias=ncol[:, rb:rb + 1])
                    u_ = work2.tile([P, N], F32, tag="geo_u")
                    nc.vector.tensor_tensor(u_[:], d_[:], ibc[:], ALU.mult)
                    l_ = work2.tile([P, N], F32, tag="geo_l")
                    nc.scalar.activation(l_[:], u_[:], AF.Ln)
                    nc.vector.tensor_scalar_max(dxy2[rb][:, ci, :], l_[:],
                                                CLIP)

        # ---------------- phase 3: dw/dh banks ----------------
        bankM = persist.tile([64, N], BF16, tag="bankM")
        bankN = persist.tile([64, N], BF16, tag="bankN")
        with tc.tile_pool(name="work3", bufs=2) as work3:
            for (pcol, bank) in ((pcolm_t, bankM), (pcoln_t, bankN)):
                t_ = work3.tile([64, N], F32, tag="bk_t")
                nc.gpsimd.tensor_scalar(t_[:], l2whbc[:], acol_t[:], pcol[:],
                                        ALU.mult, ALU.add)
                r_ = work3.tile([64, N], F32, tag="bk_r")
                nc.gpsimd.tensor_scalar(r_[:], t_[:], MAGIC, -MAGIC,
                                        ALU.add, ALU.add)
                f_ = work3.tile([64, N], F32, tag="bk_f")
                nc.gpsimd.tensor_tensor(f_[:], t_[:], r_[:], ALU.subtract)
                nc.scalar.activation(bank[:], f_[:], AF.Sin, scale=TWO_PI)

        # input casting loads then transposes (DMA xbar)
        for (src, dst) in ((xq, xq_sb), (xk, xk_sb), (xv, xv_sb)):
            nc.gpsimd.dma_start(dst[:], src.rearrange("(rb p) d -> p rb d",
                                                      p=P))
        xqTb = persist.tile([P, NRB, N], BF16, tag="xqTb")
        xkTb = persist.tile([P, NRB, N], BF16, tag="xkTb")
        xvTb = persist.tile([P, NRB, N], BF16, tag="xvTb")
        for (sb, dstb) in ((xq_sb, xqTb), (xk_sb, xkTb), (xv_sb, xvTb)):
            for rb in range(NRB):
                nc.sync.dma_start_transpose(
                    dstb[:, :, rb * P:(rb + 1) * P], sb[:, rb, :])

        qT = persist.tile([P, NRB, N], BF16, tag="qT")
        kTt = persist.tile([P, NRB, N], BF16, tag="kT")
        v_sb = persist.tile([P, NRB, D], BF16, tag="v_sb")
        upair = persist.tile([P, 8, 2, N], BF16, tag="upair")  # ((rb%2)*4+ob)
        tt_t = []
        for _rb in range(NRB):
            tt_rb = persist.tile([P, 4, 2, N], BF16, tag=f"tt_{_rb}")
            tt_t.append(tt_rb)
        ot = persist.tile([P, NRB, N], BF16, tag="ot")

        # ---------------- main loop: P4 (+proj, +P5a overlapped) ---------
        with tc.tile_pool(name="work4", bufs=3) as work4, \
             tc.tile_pool(name="work5a", bufs=3) as work5a, \
             tc.tile_pool(name="psum_u", bufs=2, space="PSUM") as psum_u, \
             tc.tile_pool(name="psum_wg", bufs=2, space="PSUM") as psum_wg:

            # software-pipelined p4: the PE stream interleaves iteration
            # g+1's selector matmuls before iteration g's WG contraction so
            # PE never blocks on the elementwise chain (and stays at full
            # clock). front() emits ups + the elementwise chain; back()
            # emits the WG matmuls + threshold + U + slab DMA.
            def p4_front(rb, g):
                off = 64 * (g // 4)
                q_ = g % 4
                ups = psum_u.tile([P, 2, N], F32, tag="ups")
                for ci in range(2):
                    nc.tensor.matmul(ups[:, ci, :],
                                     selap_t[off:off + 64, q_, :],
                                     dxy2[rb][off:off + 64, ci, :],
                                     start=True, stop=True)
                # fold: u = rn(t + MAGIC); negf = (u - MAGIC) - t
                u_ = work4.tile([P, 2, N], F32, tag="fold_u")
                if g % 2 == 0:
                    nc.vector.tensor_scalar(u_[:], ups[:], MAGIC, None,
                                            ALU.add)
                else:
                    nc.scalar.activation(u_[:], ups[:], AF.Identity,
                                         bias=magic_col[:])
                negf = work4.tile([P, 2, N], F32, tag="negf")
                nc.vector.scalar_tensor_tensor(negf[:], u_[:], MAGIC, ups[:],
                                               ALU.subtract, ALU.subtract)
                # s2 = sin(pi f), ssin = sin(2 pi f)
                s2 = work4.tile([P, 2, N], BF16, tag="s2")
                nc.scalar.activation(s2[:], negf[:], AF.Sin, scale=-PI_)
                ssin = work4.tile([P, 2, N], BF16, tag="ssin")
                nc.scalar.activation(ssin[:], negf[:], AF.Sin, scale=-TWO_PI)
                sq = work4.tile([P, 2, N], BF16, tag="sq")
                nc.gpsimd.tensor_tensor(sq[:], s2[:], s2[:], ALU.mult)
                # dw/dh lhs for this g
                lhs_wh = work4.tile([64, P], BF16, tag="lhs_wh")
                mbase = rb * P + g * GM
                nc.gpsimd.tensor_tensor(
                    lhs_wh[:].rearrange("k (h m) -> k h m", h=H),
                    w1e_f[:].rearrange("k (h m) -> k h m", h=H),
                    bankM[:, mbase:mbase + GM][:, None, :]
                        .to_broadcast((64, H, GM)),
                    ALU.mult)
                return (rb, g, ssin, sq, lhs_wh)

            def p4_back(state):
                rb, g, ssin, sq, lhs_wh = state
                kcol = rb * 8 + g
                wgp = psum_wg.tile([P, N], F32, tag="wgp")
                nc.tensor.matmul(wgp[:], wblk_t[:, 0, :], ssin[:, 0, :],
                                 start=True, stop=False)
                nc.tensor.matmul(wgp[:], wblk_t[:, 2, :], ssin[:, 1, :],
                                 start=False, stop=False)
                nc.tensor.matmul(wgp[:], lhs_wh[:], bankN[:],
                                 start=False, stop=False)
                nc.tensor.matmul(wgp[:], wblk_t[:, 1, :], sq[:, 0, :],
                                 start=False, stop=False)
                nc.tensor.matmul(wgp[:], wblk_t[:, 3, :], sq[:, 1, :],
                                 start=False, stop=True)
                # U = oj*oi*(V-1); the V-threshold step alternates ACT / DVE
                slab = work4.tile([P, N], BF16, tag="slab")
                if g % 2 == 0:
                    # A = relu(oj*wgp + oj*bG') ; U = (A - oj) * oi
                    a_ = work4.tile([P, N], BF16, tag="a_relu")
                    nc.scalar.activation(a_[:], wgp[:], AF.Relu,
                                         scale=ojp_t[:, kcol:kcol + 1],
                                         bias=ojpbg_t[:, kcol:kcol + 1])
                    nc.vector.scalar_tensor_tensor(slab[:], a_[:],
                                                   ojp_t[:, kcol:kcol + 1],
                                                   objbc[:],
                                                   ALU.subtract, ALU.mult)
                else:
                    # wgd = max(wgp + bG'-1, eps-1) ; U = (wgd * oj) * oi
                    a_ = work4.tile([P, N], BF16, tag="a_relu")
                    nc.vector.tensor_scalar(a_[:], wgp[:], bgm1_t[:],
                                            1e-6 - 1.0, ALU.add, ALU.max)
                    nc.vector.scalar_tensor_tensor(slab[:], a_[:],
                                                   ojp_t[:, kcol:kcol + 1],
                                                   objbc[:],
                                                   ALU.mult, ALU.mult)
                nc.sync.dma_start(wgd_dram[rb, :, g * GM:(g + 1) * GM, :],
                                  slab[:])

            _pipe = []

            def p4_iter(rb, g):
                _pipe.append(p4_front(rb, g))
                if len(_pipe) > 1:
                    p4_back(_pipe.pop(0))

            def p4_flush():
                while _pipe:
                    p4_back(_pipe.pop(0))

            def p5a(rb, psum_st):
                # scores -> exp -> tt = (U+1)*E, buffered in SBUF
                for ob in range(4):
                    st = psum_st.tile([P, 2, N], F32, tag="stps")
                    for hi in range(2):
                        po = hi * DK
                        nc.tensor.matmul(
                            st[:, hi, :],
                            kTt[po:po + DK, ob, rb * P:(rb + 1) * P],
                            qT[po:po + DK, ob, :], start=True, stop=True)
                    e_ = work5a.tile([P, 2, N], BF16, tag="e_")
                    nc.scalar.activation(e_[:], st[:],
                                         AF.Exp, bias=mcol_t[:, rb:rb + 1])
                    nc.vector.scalar_tensor_tensor(
                        tt_t[rb][:, ob, :, :],
                        upair[:, (rb % 2) * 4 + ob, :, :], 1.0, e_[:],
                        ALU.add, ALU.mult)

            # ---- rb 0 ----  (x/weight cast loads emitted mid-rb0 so the
            # Pool queue serves the first sq ops and const DMA transfers
            # are not delayed by the big input transfers)
            _wload = {3: (Wq, wq_b), 4: (Wk, wk_b), 5: (Wv, wv_b),
                      6: (Wo, wo_b)}
            for g in range(NG):
                p4_iter(0, g)
                if g in _wload:
                    Wd, wb_ = _wload[g]
                    nc.gpsimd.dma_start(
                        wb_[:], Wd.rearrange("(kb p) d -> p kb d", p=P))
            with tc.tile_pool(name="psum_pr", bufs=2, space="PSUM") as ppr, \
                 tc.tile_pool(name="workp", bufs=2) as workp:
                for (wb_, xb, dstT, bcol, scl) in (
                        (wq_b, xqTb, qT, bq_t, 1.0),
                        (wk_b, xkTb, kTt, bk8_t, 0.125)):
                    for ob in range(NRB):
                        ps = ppr.tile([P, N], F32, tag="projps")
                        for kb in range(NRB):
                            nc.tensor.matmul(ps[:],
                                             wb_[:, kb, ob * P:(ob + 1) * P],
                                             xb[:, kb, :],
                                             start=(kb == 0),
                                             stop=(kb == NRB - 1))
                        nc.scalar.activation(dstT[:, ob, :], ps[:],
                                             AF.Identity, scale=scl,
                                             bias=bcol[:, ob:ob + 1])
                for mb in range(NRB):
                    ps = ppr.tile([P, D], F32, tag="projps")
                    for kb in range(NRB):
                        nc.tensor.matmul(ps[:],
                                         xvTb[:, kb, mb * P:(mb + 1) * P],
                                         wv_b[:, kb, :],
                                         start=(kb == 0), stop=(kb == NRB - 1))
                    nc.vector.tensor_tensor(v_sb[:, mb, :], ps[:], bvbc[:],
                                            ALU.add)
            p4_flush()
            for hp in range(4):
                nc.sync.dma_start(
                    upair[:, hp, :, :],
                    wgd_dram[0, 2 * hp:2 * hp + 2, :, :]
                        .rearrange("h p i -> p h i"))

            # ---- rb 1..3 ----
            with tc.tile_pool(name="psum_st", bufs=1, space="PSUM") as pst, \
                 tc.tile_pool(name="work5", bufs=2) as work5:
                for rb in range(1, NRB):
                    for g in range(NG):
                        p4_iter(rb, g)
                    p4_flush()
                    for hp in range(4):
                        nc.sync.dma_start(
                            upair[:, (rb % 2) * 4 + hp, :, :],
                            wgd_dram[rb, 2 * hp:2 * hp + 2, :, :]
                                .rearrange("h p i -> p h i"))
                    p5a(rb - 1, pst)
                p5a(NRB - 1, pst)
                # ------- phase 5b: pure PE burst over buffered tt -------
                # av packed per head-pair-pair into [P,2,N] tiles reusing the
                # ups pool's banks; sums reuse the wgp pool's banks.
                av01 = psum_u.tile([P, 2, N], F32, tag="ups")
                av23 = psum_u.tile([P, 2, N], F32, tag="ups")
                av_sl = [(av01, 0), (av01, 1), (av23, 0), (av23, 1)]
                sm_ps = []
                for pr in range(2):
                    sm_t = psum_wg.tile([P, N], F32, tag="wgp")
                    sm_ps.append(sm_t)
                for pr in range(2):
                    for ob in (2 * pr, 2 * pr + 1):
                        av_t, sl = av_sl[ob]
                        for rb in range(NRB):
                            s_ = rb * 4 + ob
                            for hi in range(2):
                                h = 2 * ob + hi
                                po = hi * DK
                                nc.tensor.matmul(
                                    sm_ps[pr][0:8, :], oh8_t[:, h, :],
                                    tt_t[rb][:, ob, hi, :],
                                    start=(ob == 2 * pr and rb == 0
                                           and hi == 0),
                                    stop=(ob == 2 * pr + 1 and rb == NRB - 1
                                          and hi == 1),
                                    skip_group_check=True)
                                nc.tensor.matmul(
                                    av_t[po:po + DK, sl, :],
                                    v_sb[:, rb, h * DK:(h + 1) * DK],
                                    tt_t[rb][:, ob, hi, :],
                                    start=(rb == 0),
                                    stop=(rb == NRB - 1),
                                    skip_group_check=True)
                    rs = work5.tile([H, N], F32, tag="rs")
                    nc.vector.reciprocal(rs[:], sm_ps[pr][0:8, :])
                    nc.sync.dma_start(rs_dram[4 * pr:4 * pr + 4, :],
                                      rs[4 * pr:4 * pr + 4, :])
                    for ob in (2 * pr, 2 * pr + 1):
                        av_t, sl = av_sl[ob]
                        rr_b = work5.tile([P, N], F32, tag="rr_b")
                        for hi in range(2):
                            nc.sync.dma_start(
                                rr_b[hi * DK:(hi + 1) * DK, :],
                                rs_dram[2 * ob + hi:2 * ob + hi + 1, :]
                                    .to_broadcast((DK, N)))
                        nc.vector.tensor_tensor(ot[:, ob, :],
                                                av_t[:, sl, :],
                                                rr_b[:], ALU.mult)
                # ------- phase 6: final linear (psum from the ups pool) ----
                for r in range(NRB):
                    ps = psum_u.tile([P, 2, N], F32, tag="ups")
                    for kt in range(NRB):
                        nc.tensor.matmul(ps[:, 0, :],
                                         ot[:, kt, r * P:(r + 1) * P],
                                         wo_b[:, kt, :],
                                         start=(kt == 0), stop=(kt == NRB - 1))
                    fo = work5.tile([P, D], F32, tag="fo")
                    nc.vector.tensor_tensor(fo[:], ps[:, 0, :], bobc[:],
                                            ALU.add)
                    nc.sync.dma_start(out[r * P:(r + 1) * P, :], fo[:])

    _fix_waits(nc)
    return nc


_NC_CACHE = {}


def kernel(**inputs):
    in_maps = _host_prep(inputs)
    if "nc" not in _NC_CACHE:
        _NC_CACHE["nc"] = build_nc()
    nc = _NC_CACHE["nc"]
    res = run_bass_kernel_spmd(nc, in_maps, list(range(B)))
    out = np.stack([res.results[b]["out"] for b in range(B)], axis=0)
    return out.astype(np.float32)


if __name__ == "__main__":
    print("kernel module ok")
